# revision 1
# baseline (speedup 1.0000x reference)
"""Trainium2 Bass kernel for 7x7 local (sparse) attention, SPMD over 8 NeuronCores.

Math (per channel c, pixel p):
    q = w_q @ x, k = w_k @ x            (1x1 convs)
    logit[c,p,(i,j)] = q[c,p] * (kpad[c, p+(i,j)] + rel[c,(i,j)])
    out[c,p] = sum_k softmax_k(logit) * vpad[c, p+(i,j)]
where rel[c,(i,j)] = rel_h[c,i] for c<64, rel_w[c-64,j] for c>=64, and
kpad/vpad are zero-padded by 3 (softmax runs over all 49 taps, padded taps
contribute exp(q*rel) to the denominator -- matching the reference exactly).

Sharding: 8 cores = 2 batches x 4 H-tiles of 24 rows (+3-row halo each side,
zero-filled at image edges on the host so all cores run one SPMD graph).

Per-core device pipeline (processed as 14 groups = 2 pixel-epochs x 7 kw):
  DVE: per offset tensor_scalar (kbuf window + rel[c]) -> krel fp16 (4x mode);
       one batched tensor_tensor per group for logits (q broadcast over the 7
       kh via a stride-0 AP) and one for e*v (overlapping-row v AP), 2x mode.
  ACT: one exp per group (7 offsets batched), 3 of 7 krel adds per group
       (engine balancing), and the final recip = exp(-ln(den)).
  PE:  q/k 1x1 convs, then one identity-matmul per (offset, 256-px chunk)
       accumulating [num | den] jointly into 5 PSUM banks per epoch.
The emission is software-pipelined by one group so ScalarE's exp[g] overlaps
group g+1's krel build (otherwise ACT and DVE ping-pong serially).
Output epoch = 12 rows x 96 cols (PSUM capacity forces 2 epochs).

fp16 is used for the whole logit chain (q, kbuf, krel, logit): bf16's 8-bit
mantissa on |logit|<=60 costs ~4% absmax error; fp16 keeps it ~1%.
e/prod are bf16 (need range: e up to exp(60)).
"""

import sys
import types

import numpy as np
import ml_dtypes

sys.path.insert(0, "/opt/trn_rl_repo")

import concourse.bass as bass  # noqa: E402
import concourse.tile as tile  # noqa: E402
from concourse import mybir  # noqa: E402
from concourse.vector_clock import ScopedClock  # noqa: E402

F32 = mybir.dt.float32
BF16 = mybir.dt.bfloat16
FP16 = mybir.dt.float16

B, CIN, COUT, H, W, K, PAD = 2, 128, 128, 96, 96, 7, 3
NCORES = 8
HT = 4           # H tiles per batch
TROWS = H // HT  # 24 rows per core
SLAB = TROWS + 2 * PAD  # 30 rows incl halo
WPAD = 104       # 96 + 6 pad + 2 alignment spare
EP_ROWS = 12     # rows per PSUM epoch
EP = TROWS // EP_ROWS  # 2 epochs
FD = EP_ROWS * W       # 1152
# PSUM chunking: each bank holds [num(<=256 px) | den(same px)] so one matmul
# accumulates both. 1152 px = 4 chunks of 256 + 1 of 128 -> 5 banks, 5 MMs.
CHUNKS = [(0, 256), (256, 256), (512, 256), (768, 256), (1024, 128)]
ACT_KREL_I = (1, 3, 5)  # i-values whose krel runs on ScalarE (engine balance)

_cache = {}


def _patch_tile_drain():
    """walrus in this container allows only one sync-wait per instruction;
    split excess waits onto NoOps."""
    if getattr(tile, "_drain_patched", False):
        return

    def _drain_and_barrier(self, tick_clock, wait_clock):
        drain_inst = self.nc.sync.drain()
        wait_clock.add_sem_waits(
            drain_inst.ins, ScopedClock({None: tick_clock.global_clock})
        )
        si = drain_inst.ins.sync_info
        if si is not None and si.on_wait and len(si.on_wait) > 1:
            waits = list(si.on_wait)
            drain_inst.ins.sync_info = mybir.SyncInfo(
                on_wait=waits[:1], on_update=list(si.on_update)
            )
            for w in waits[1:]:
                nop_inst = self.nc.sync.nop()
                nop_inst.ins.sync_info = mybir.SyncInfo(on_wait=[w], on_update=[])
        self.nc.all_engine_barrier()
        assert self.sems is not None
        popped = self.nc._tile_sem_poison_stack.pop()
        assert popped is self._sem_poison
        self.nc.clear_and_free_semaphores(list(self.sems.allocated().values()))
        self.nc.all_engine_barrier()

    tile.TileContext._drain_and_barrier = _drain_and_barrier
    tile._drain_patched = True


_split_ctr = [0]


def _split_sync_waits(nc, maxw=1):
    for fn in nc.m.functions:
        for bb in fn.blocks:
            if not any(
                inst.sync_info is not None
                and inst.sync_info.on_wait
                and len(inst.sync_info.on_wait) > maxw
                for inst in bb.instructions
            ):
                continue
            new_list = []
            for inst in bb.instructions:
                si = inst.sync_info
                if si is not None and si.on_wait and len(si.on_wait) > maxw:
                    waits = list(si.on_wait)
                    extra, keep = waits[:-maxw], waits[-maxw:]
                    for i in range(0, len(extra), maxw):
                        _split_ctr[0] += 1
                        nop = mybir.InstNoOp(
                            name=f"splitw-{_split_ctr[0]}", ins=[], outs=[]
                        )
                        nop.engine = inst.engine
                        nop.sync_info = mybir.SyncInfo(
                            on_wait=extra[i : i + maxw], on_update=[]
                        )
                        new_list.append(nop)
                    inst.sync_info = mybir.SyncInfo(
                        on_wait=keep, on_update=list(si.on_update)
                    )
                new_list.append(inst)
            try:
                bb.instructions = new_list
            except Exception:
                bb.instructions.clear()
                bb.instructions.extend(new_list)


def _build():
    _patch_tile_drain()
    nc = bass.Bass("TRN2", target_bir_lowering=False, debug=False)

    xs_ext = nc.dram_tensor("xs", [128, SLAB * W], FP16, kind="ExternalInput").ap()
    va_ext = nc.dram_tensor("vbufA", [128, SLAB, WPAD], BF16, kind="ExternalInput").ap()
    vb_ext = nc.dram_tensor("vbufB", [128, SLAB, WPAD], BF16, kind="ExternalInput").ap()
    wq_ext = nc.dram_tensor("wqT", [128, 128], FP16, kind="ExternalInput").ap()
    wk_ext = nc.dram_tensor("wkT", [128, 128], FP16, kind="ExternalInput").ap()
    rel_ext = nc.dram_tensor("relcols", [128, K * K], F32, kind="ExternalInput").ap()
    id_ext = nc.dram_tensor("ident", [128, 128], BF16, kind="ExternalInput").ap()
    out_ext = nc.dram_tensor("out", [128, TROWS * W], F32, kind="ExternalOutput").ap()

    from contextlib import ExitStack

    with tile.TileContext(nc) as tc, ExitStack() as ctx:
        consts = ctx.enter_context(tc.tile_pool(name="consts", bufs=1))
        main = ctx.enter_context(tc.tile_pool(name="main", bufs=1))
        krelp = ctx.enter_context(tc.tile_pool(name="krelp", bufs=3))
        ep_pool = ctx.enter_context(tc.tile_pool(name="ep", bufs=3))
        outp = ctx.enter_context(tc.tile_pool(name="outp", bufs=1))

        wqT = consts.tile([128, 128], FP16)
        wkT = consts.tile([128, 128], FP16)
        ident = consts.tile([128, 128], BF16)
        relc = consts.tile([128, K * K], F32)
        xs = main.tile([128, SLAB * W], FP16)
        # xs alone on the sync queue (it gates the k matmuls); weights on the
        # vector queue so they land in parallel; bulk v + consts on scalar.
        nc.sync.dma_start(out=xs, in_=xs_ext)
        nc.scalar.dma_start(out=wkT, in_=wk_ext)
        nc.scalar.dma_start(out=wqT, in_=wq_ext)
        vbufA = main.tile([128, SLAB, WPAD], BF16)
        vbufB = main.tile([128, SLAB, WPAD], BF16)
        nc.scalar.dma_start(out=relc, in_=rel_ext)
        nc.scalar.dma_start(out=ident, in_=id_ext)
        nc.scalar.dma_start(out=vbufA, in_=va_ext)
        nc.scalar.dma_start(out=vbufB, in_=vb_ext)

        # ---- preamble: q = wq.T @ xs, k = wk.T @ xs (2880 px in 6x480 chunks)
        NPRE = 6
        PREW = SLAB * W // NPRE  # 480
        q_sb = main.tile([128, SLAB * W], FP16)
        kbufA = main.tile([128, SLAB, WPAD], FP16)
        kbufB = main.tile([128, SLAB, WPAD], FP16)
        # only the pad columns need zeroing (interior is overwritten below)
        nc.vector.memset(kbufA[:, :, 0:3], 0.0)
        nc.vector.memset(kbufA[:, :, 3 + W : WPAD], 0.0)
        nc.vector.memset(kbufB[:, :, 0:2], 0.0)
        nc.vector.memset(kbufB[:, :, 2 + W : WPAD], 0.0)
        with tc.tile_pool(name="ps_pre", bufs=1, space="PSUM") as ps_pre:
            # k first: it gates the whole offset loop (q only gates logits)
            k_ps = ps_pre.tile([128, NPRE, 512], F32, tag="pre")
            for c in range(NPRE):
                nc.tensor.matmul(
                    k_ps[:, c, 0:PREW], wkT, xs[:, c * PREW : (c + 1) * PREW],
                    start=True, stop=True,
                )
            # kf chunk c covers rows 5c..5c+4; write into the padded buffers
            k_src = k_ps[:, :, 0:PREW].rearrange("p c (r w) -> p c r w", r=5)
            kA_view = kbufA[:, :, 3 : 3 + W].rearrange(
                "p (c r) w -> p c r w", c=NPRE
            )
            kB_view = kbufB[:, :, 2 : 2 + W].rearrange(
                "p (c r) w -> p c r w", c=NPRE
            )
            nc.vector.tensor_copy(kA_view, k_src)
            nc.scalar.copy(out=kB_view, in_=k_src)
            q_ps = ps_pre.tile([128, NPRE, 512], F32, tag="pre")
            for c in range(NPRE):
                nc.tensor.matmul(
                    q_ps[:, c, 0:PREW], wqT, xs[:, c * PREW : (c + 1) * PREW],
                    start=True, stop=True,
                )
            nc.vector.tensor_copy(
                q_sb.rearrange("p (c w) -> p c w", c=NPRE), q_ps[:, :, 0:PREW]
            )

        # ---- main loop: 2 epochs x 7 j-groups x 7 i, software-pipelined by
        # one j-group so ScalarE's EXP[g] runs while group g+1's krel builds
        # (otherwise A and V ping-pong: ID[g] -> logit[g] -> EXP[g] -> ID[g+1]).
        ps_loop = ctx.enter_context(tc.tile_pool(name="ps_loop", bufs=1, space="PSUM"))

        nd_tiles = {}

        def phase_krel(e, j):
            """krel[i] = (kbuf window + rel) then logit = q * krel (all 7 i)."""
            P = j & 1
            j2 = j - P
            kbuf = kbufB if P else kbufA
            krel = krelp.tile([128, K, FD], FP16, tag="krel")
            for i in range(K):
                r0 = i + EP_ROWS * e
                ksrc = kbuf[:, r0 : r0 + EP_ROWS, j2 : j2 + W]
                kdst = krel[:, i, :].rearrange("p (r w) -> p r w", r=EP_ROWS)
                rel_col = relc[:, i * K + j : i * K + j + 1]
                if i in ACT_KREL_I:
                    nc.scalar.activation(
                        out=kdst, in_=ksrc,
                        func=mybir.ActivationFunctionType.Identity,
                        bias=rel_col, scale=1.0,
                    )
                else:
                    nc.vector.tensor_scalar(
                        out=kdst, in0=ksrc, scalar1=rel_col, scalar2=None,
                        op0=mybir.AluOpType.add,
                    )
            qs = q_sb[:, (EP_ROWS * e + PAD) * W : (EP_ROWS * e + PAD) * W + FD]
            q_bc = bass.AP(qs.tensor, qs.offset, [qs.ap[0], [0, K], qs.ap[1]])
            nc.vector.tensor_tensor(
                out=krel, in0=q_bc, in1=krel, op=mybir.AluOpType.mult
            )
            return krel

        def phase_rest(e, j, krel, split=False):
            """exp, e*v, and the accumulate matmuls for group (e, j). With
            split=True (last group of an epoch) run in two i-halves so the
            tail matmuls/normalize start earlier."""
            P = j & 1
            j2 = j - P
            vbuf = vbufB if P else vbufA
            ept = ep_pool.tile([128, K, 2, FD], BF16, tag="ept")
            if j == 0:
                nd_tiles[e] = ps_loop.tile([128, len(CHUNKS), 512], F32, tag="nd", name=f"nd{e}")
            nd_ps = nd_tiles[e]
            rowstep = vbuf.ap[1][0]
            halves = [(0, 4), (4, K)] if split else [(0, K)]
            for i0, i1 in halves:
                ni = i1 - i0
                nc.scalar.activation(
                    out=ept[:, i0:i1, 1, :], in_=krel[:, i0:i1, :],
                    func=mybir.ActivationFunctionType.Exp, bias=0.0, scale=1.0,
                )
                vbase = vbuf[:, EP_ROWS * e + i0 : EP_ROWS * e + i0 + 1, j2 : j2 + W]
                v_ov = bass.AP(
                    vbase.tensor, vbase.offset,
                    [vbase.ap[0], [rowstep, ni], [rowstep, EP_ROWS], [1, W]],
                )
                nc.vector.tensor_tensor(
                    out=ept[:, i0:i1, 0, :].rearrange("p k (r w) -> p k r w", r=EP_ROWS),
                    in0=ept[:, i0:i1, 1, :].rearrange("p k (r w) -> p k r w", r=EP_ROWS),
                    in1=v_ov, op=mybir.AluOpType.mult,
                )
                for i in range(i0, i1):
                    for c, (px0, cw) in enumerate(CHUNKS):
                        nc.tensor.matmul(
                            nd_ps[:, c, 0 : 2 * cw], ident,
                            ept[:, i, :, px0 : px0 + cw],
                            start=(j == 0 and i == 0), stop=(j == K - 1 and i == K - 1),
                        )

        def normalize(e):
            """out = num * exp(-ln(den)); den chunks are 4x256 + 1x128."""
            nd_ps = nd_tiles[e]
            lnden = outp.tile([128, FD], F32, tag="lnden")
            ln4 = lnden[:, 0:1024].rearrange("p (c w) -> p c w", c=4)
            nc.scalar.activation(
                out=ln4, in_=nd_ps[:, 0:4, 256:512],
                func=mybir.ActivationFunctionType.Ln, bias=0.0, scale=1.0,
            )
            nc.scalar.activation(
                out=lnden[:, 1024:FD], in_=nd_ps[:, 4, 128:256],
                func=mybir.ActivationFunctionType.Ln, bias=0.0, scale=1.0,
            )
            recip = outp.tile([128, FD], F32, tag="recip")
            nc.scalar.activation(
                out=recip, in_=lnden,
                func=mybir.ActivationFunctionType.Exp, bias=0.0, scale=-1.0,
            )
            out_sb = outp.tile([128, FD], F32, tag="out_sb")
            nc.vector.tensor_tensor(
                out=out_sb[:, 0:1024].rearrange("p (c w) -> p c w", c=4),
                in0=nd_ps[:, 0:4, 0:256],
                in1=recip[:, 0:1024].rearrange("p (c w) -> p c w", c=4),
                op=mybir.AluOpType.mult,
            )
            nc.vector.tensor_tensor(
                out=out_sb[:, 1024:FD], in0=nd_ps[:, 4, 0:128],
                in1=recip[:, 1024:FD], op=mybir.AluOpType.mult,
            )
            nc.sync.dma_start(out=out_ext[:, e * FD : (e + 1) * FD], in_=out_sb)

        groups = [(e, j) for e in range(EP) for j in range(K)]
        pending = None  # (e, j, krel) whose exp/prod/MMs are not yet emitted
        for e, j in groups:
            krel = phase_krel(e, j)
            if pending is not None:
                pe, pj, pkrel = pending
                phase_rest(pe, pj, pkrel, split=(pj == K - 1))
                if pj == K - 1:
                    normalize(pe)
            pending = (e, j, krel)
        pe, pj, pkrel = pending
        phase_rest(pe, pj, pkrel, split=True)
        normalize(pe)

    _split_sync_waits(nc)
    return nc


def _host_prep(x, v, w_q, w_k, rel_h, rel_w):
    """Build the 8 per-core input maps (numpy only)."""
    x = np.asarray(x, np.float32)
    v = np.asarray(v, np.float32)
    w_q = np.asarray(w_q, np.float32)
    w_k = np.asarray(w_k, np.float32)
    rel_h = np.asarray(rel_h, np.float32)
    rel_w = np.asarray(rel_w, np.float32)

    wqT = np.ascontiguousarray(w_q.T).astype(np.float16)
    wkT = np.ascontiguousarray(w_k.T).astype(np.float16)
    ident = np.eye(128, dtype=np.float32).astype(ml_dtypes.bfloat16)
    relc = np.zeros((128, K * K), np.float32)
    for i in range(K):
        for j in range(K):
            relc[:64, i * K + j] = rel_h[:, 0, 0, i, 0]
            relc[64:, i * K + j] = rel_w[:, 0, 0, 0, j]

    # v padded: rows with halo, cols padded by 3 (plus alignment spare)
    vpad = np.zeros((B, COUT, H + 2 * PAD, WPAD), np.float32)
    vpad[:, :, PAD : PAD + H, PAD : PAD + W] = v

    in_maps = []
    for ci in range(NCORES):
        b, ht = divmod(ci, HT)
        r0 = ht * TROWS  # first output row of this tile
        xs = np.zeros((128, SLAB, W), np.float32)
        glo = max(0, r0 - PAD)
        ghi = min(H, r0 + TROWS + PAD)
        xs[:, glo - (r0 - PAD) : ghi - (r0 - PAD), :] = x[b, :, glo:ghi, :]
        vslab = vpad[b, :, r0 : r0 + SLAB, :]  # rows r0-3..r0+26 in orig coords
        vbufA = vslab.astype(ml_dtypes.bfloat16)
        vbufB = np.zeros_like(vslab)
        vbufB[:, :, : WPAD - 1] = vslab[:, :, 1:]
        in_maps.append(
            {
                "xs": np.ascontiguousarray(xs.reshape(128, SLAB * W)).astype(np.float16),
                "vbufA": np.ascontiguousarray(vbufA),
                "vbufB": np.ascontiguousarray(vbufB.astype(ml_dtypes.bfloat16)),
                "wqT": wqT,
                "wkT": wkT,
                "relcols": relc,
                "ident": ident,
            }
        )
    return in_maps


def kernel(x, v, w_q, w_k, rel_h, rel_w, trace=False, tmpdir=None):
    from concourse.bass_utils import run_bass_kernel_spmd

    if "nc" not in _cache:
        _cache["nc"] = _build()
    nc = _cache["nc"]
    in_maps = _host_prep(x, v, w_q, w_k, rel_h, rel_w)
    res = run_bass_kernel_spmd(
        nc, in_maps, list(range(NCORES)), trace=trace, tmpdir=tmpdir
    )
    out = np.zeros((B, COUT, H, W), np.float32)
    for ci in range(NCORES):
        b, ht = divmod(ci, HT)
        out[b, :, ht * TROWS : (ht + 1) * TROWS, :] = (
            res.results[ci]["out"].reshape(128, TROWS, W)
        )
    kernel.last_exec_time_ns = res.exec_time_ns
    kernel.last_results = res
    return out

